# revision 1
# baseline (speedup 1.0000x reference)
"""Trainium2 Bass kernel for nn_MinBlcokScan: 4 grouped 1-D cross-correlations.

Math (reference): x = batch_x.reshape(B, 32, L). For each group g of 4,
channels rel_g = [8g..8g+7] are convolved ('same', zero pad 2/2) with
kernels_g [4, 8, 5], producing out[:, 4g+o, :]; the 16 output channels are
concatenated and flattened to [B, 16*L].

Strategy: pure data parallel over batch (4 samples per core) plus a
polyphase-2 reformulation that packs two L-positions per streamed PE
column, cutting TensorEngine column count from 5L to 3L.

Host-side marshalling (free for the device):
  x is zero-padded by 2 each side and parity-interleaved:
    x_i[(s, c, p), m] = x_pad[s, c, 2m + p],  m in [0, L/2+2)
  so one SBUF partition column m carries both parities for 2 samples x 32
  channels = 128 contraction rows. The conv becomes 3 PSUM-accumulated
  matmuls per output tile, with window offsets d in {-1,0,+1}:
    y[s, o, 2m+r] = sum_d (W_d.T @ x_i[:, m+d])[(s,o,r)]
    W_d[(s,c,p), (s,o,r)] = ker[o, c, t],  t = 2d + p + 2 - r  (valid t only)
  block-diagonal over the 2 samples of a pass; 2 passes cover 4 samples.
  Output is produced parity-interleaved y_i[(s, o, r), m] = y[s, o, 2m+r]
  and de-interleaved on the host.

Matmuls run in float32r (full-rate fp32 PE mode, fp32 PSUM accumulate),
so accuracy is ~1e-4 relative.
"""

import numpy as np
from contextlib import ExitStack

import concourse.bass as bass
import concourse.bacc as bacc
import concourse.mybir as mybir
import concourse.tile as tile
from concourse.bass_utils import run_bass_kernel_spmd

D = 32          # input channels
L_FULL = 65536  # sequence length
W = 5           # conv window
PAD = 2         # left zero-pad ('same')
B = 32          # batch
N_CORES = 8
S = 4           # samples per core
NSUB = 512      # matmul moving free dim == one fp32 PSUM bank
ND = 3          # window offsets d in {-1, 0, 1}
F32 = mybir.dt.float32
F32R = mybir.dt.float32r


def build_program(L=L_FULL, blk_m=2048, reps=1, variant="full"):
    """Build the single-core SPMD Bass program (same program on all cores).

    blk_m: per-block m-columns (= blk_m*2 L positions).
    reps > 1 wraps the body in a hardware For_i loop (steady-state timing).
    variant: "full" | "dma" (loads+stores only) | "pe" (loads+matmuls only)
    """
    M = L // 2  # m-columns total
    assert M % blk_m == 0 and blk_m % NSUB == 0
    nblk = M // blk_m
    nq = blk_m // NSUB

    nc = bacc.Bacc(trn_type="TRN2", target_bir_lowering=False, debug=False)
    x = nc.dram_tensor("x", [2 * 128, M + 2], F32R, kind="ExternalInput").ap()
    w = nc.dram_tensor("w", [ND, 128, 64], F32R, kind="ExternalInput").ap()
    y = nc.dram_tensor("y", [128, M], F32, kind="ExternalOutput").ap()

    with tile.TileContext(nc) as tc, ExitStack() as ctx:
        xp = ctx.enter_context(tc.tile_pool(name="xp", bufs=4))
        wp = ctx.enter_context(tc.tile_pool(name="wp", bufs=1))
        op = ctx.enter_context(tc.tile_pool(name="op", bufs=3))
        pp = ctx.enter_context(tc.tile_pool(name="pp", bufs=8, space="PSUM"))

        # Load the 3 offset-weight matrices once: wt[:, d*64 + mcol] = w[d, :, mcol]
        wt = wp.tile([128, ND * 64], F32R)
        nc.sync.dma_start(
            wt[:].rearrange("p (d m) -> p d m", d=ND),
            w.rearrange("d p m -> p d m"),
        )

        if reps > 1:
            loop_cm = tc.For_i(
                0, reps, 1,
                hint_engines=(mybir.EngineType.PE, mybir.EngineType.DVE,
                              mybir.EngineType.SP, mybir.EngineType.Activation),
            )
            ctx.enter_context(loop_cm)

        for b in range(nblk):
            m0 = b * blk_m
            ot = None
            if variant != "pe":
                ot = op.tile([128, blk_m], F32)
            if variant == "dma":
                nc.vector.memset(ot[:], 0.0)

            for ps in range(2):  # sample-pair pass: samples (2ps, 2ps+1)
                xt = xp.tile([128, blk_m + 2], F32R)
                nc.sync.dma_start(xt[:], x[128 * ps : 128 * (ps + 1), m0 : m0 + blk_m + 2])

                if variant == "dma":
                    continue
                for q in range(nq):
                    pt = pp.tile([64, NSUB], F32)
                    for d in range(ND):
                        nc.tensor.matmul(
                            pt[:],
                            wt[:, d * 64 : (d + 1) * 64],
                            xt[:, q * NSUB + d : q * NSUB + d + NSUB],
                            start=(d == 0),
                            stop=(d == ND - 1),
                        )
                    if variant == "full":
                        # partition-shifted PSUM->SBUF copy (ps=1 -> 64:128)
                        nc.vector.tensor_copy(
                            ot[ps * 64 : (ps + 1) * 64, q * NSUB : (q + 1) * NSUB],
                            pt[:],
                        )

            if variant != "pe":
                nc.scalar.dma_start(y[:, m0 : m0 + blk_m], ot[:])
    nc.compile()
    return nc


def build_weights(kernels):
    """W_d [3, 128, 64]: W_d[(s,c,p), (s,o,r)] = ker_g[o, c, t], t = 2d+p+2-r.

    s in {0,1} is the sample within a pass (block-diagonal), c channel,
    p source parity, o output channel (16 = 4 groups x 4), r output parity.
    """
    Wd = np.zeros((ND, 128, 64), np.float32)
    for g, ker in enumerate(kernels):  # ker [4, 8, 5]
        for o_in_g in range(4):
            o = 4 * g + o_in_g
            for c_in_g in range(8):
                c = 8 * g + c_in_g
                for r in range(2):
                    for t in range(W):
                        dd = (r + t - 2) >> 1  # floor((r+t-2)/2)
                        p = (r + t - 2) - 2 * dd
                        assert -1 <= dd <= 1
                        for s in range(2):
                            Wd[dd + 1, s * 64 + c * 2 + p, s * 32 + o * 2 + r] = \
                                ker[o_in_g, c_in_g, t]
    return Wd


def interleave_x(x4, L):
    """[4, 32, L] -> [256, L/2+2]: row (s*64 + c*2 + p), col m = x_pad[s,c,2m+p]."""
    xp = np.zeros((4, D, L + 4), np.float32)
    xp[:, :, 2 : L + 2] = x4
    xi = xp.reshape(4, D, (L + 4) // 2, 2).transpose(0, 1, 3, 2)  # s, c, p, m
    return np.ascontiguousarray(xi.reshape(256, (L + 4) // 2))


def deinterleave_y(yi, L):
    """[128, L/2] -> [64, L]: yi[s*32+o*2+r, m] = y[s*16+o, 2m+r]."""
    t = yi.reshape(4, 16, 2, L // 2).transpose(0, 1, 3, 2)  # s, o, m, r
    return np.ascontiguousarray(t.reshape(64, L))


_program_cache = {}

# Set PROFILE=True (e.g. from a test harness) to capture an NTFF profile;
# the BassKernelResults lands in LAST_RESULT.
PROFILE = False
LAST_RESULT = None


def kernel(batch_x, kernels0, kernels1, kernels2, kernels3):
    global LAST_RESULT
    batch_x = np.asarray(batch_x)
    kernels = [np.asarray(k) for k in (kernels0, kernels1, kernels2, kernels3)]
    Wd = build_weights(kernels)

    if "nc" not in _program_cache:
        _program_cache["nc"] = build_program()
    nc = _program_cache["nc"]

    in_maps = [
        {
            "x": interleave_x(
                batch_x[S * k : S * (k + 1)].reshape(S, D, L_FULL), L_FULL
            ),
            "w": Wd,
        }
        for k in range(N_CORES)
    ]
    res = run_bass_kernel_spmd(nc, in_maps, list(range(N_CORES)), trace=PROFILE)
    LAST_RESULT = res
    ys = [deinterleave_y(res.results[k]["y"], L_FULL) for k in range(N_CORES)]
    return np.concatenate(ys, axis=0).reshape(B, 16 * L_FULL)



# revision 13
# speedup vs baseline: 1.5886x; 1.5886x over previous
"""Trainium2 Bass kernel for nn_MinBlcokScan: 4 grouped 1-D cross-correlations.

Math (reference): x = batch_x.reshape(B, 32, L). For each group g of 4,
channels [8g..8g+7] are convolved ('same', zero pad 2/2) with kernels_g
[4, 8, 5] producing out[:, 4g+o, :]; outputs concatenated to [B, 16*L].

Strategy: pure data parallel over batch (4 samples per core) plus a
polyphase-16 reformulation in bf16. For each (group g, sample s) = u, the
host packs x into a [128, L/16 + 2] tile whose partitions are (phase, chan).
Per 512-column PSUM block and sample-pair, six independent 128-row matmuls
(each start=stop=True — no in-bank accumulation chains, which this runtime
serializes and miscompiles at mixed row offsets) compute:

  bank A: main taps (in-column phases)   -> [0:64] sample0, [64:128] sample1
  bank D: cross-column taps, using 32-wide weight slabs whose only nonzero
          columns are the r{0,1} (left, x view shifted -1) and r{14,15}
          (right, +1) output phases -> 32-aligned regions mirroring bank A

One Vector/Pool tensor-tensor add per block (ot = A + D, cast to bf16)
replaces the PSUM->SBUF copy, so the edge combine costs no extra engine
time. All DMA moves 128-partition tiles with >=8KB contiguous descriptors
(this runtime charges narrow-partition DMAs at full 128-partition price).
I/O is bf16 (fp32 PSUM accumulation), rel err ~3e-3 vs the fp32 reference.
"""

import numpy as np
from contextlib import ExitStack

import ml_dtypes

import concourse.bass as bass
import concourse.bacc as bacc
import concourse.mybir as mybir
import concourse.tile as tile
from concourse.bass_utils import run_bass_kernel_spmd

D = 32           # input channels
L = 65536        # sequence length
W = 5            # conv window
B = 32           # batch
N_CORES = 8
S = 4            # samples per core
M = L // 16      # polyphase columns per (g, s) = 4096
J = M + 2        # zero-padded columns in the x tile
NSUB = 512       # matmul moving free dim == one fp32 PSUM bank
NQ = M // NSUB   # 8 column blocks per u

F32 = mybir.dt.float32
BF16 = mybir.dt.bfloat16
NP_BF16 = ml_dtypes.bfloat16

# phase permutation kept from the earlier layout (any layout works for this
# design; PERM[0..1]=0..1 etc. retained for marshalling stability)
PERM = {0: 0, 1: 1, 2: 2, 3: 3, 14: 4, 15: 5}
for _ph in range(4, 14):
    PERM[_ph] = _ph + 2
INVPERM = [0] * 16
for _ph, _p in PERM.items():
    INVPERM[_p] = _ph


def build_program():
    nc = bacc.Bacc(trn_type="TRN2", target_bir_lowering=False, debug=False)
    x = nc.dram_tensor("x", [16 * 128, J], BF16, kind="ExternalInput").ap()
    wm = nc.dram_tensor("wm", [128, 256], BF16, kind="ExternalInput").ap()
    wl = nc.dram_tensor("wl", [128, 128], BF16, kind="ExternalInput").ap()
    wr = nc.dram_tensor("wr", [128, 128], BF16, kind="ExternalInput").ap()
    y = nc.dram_tensor("y", [16 * 64, M], BF16, kind="ExternalOutput").ap()

    with tile.TileContext(nc) as tc, ExitStack() as ctx:
        wp = ctx.enter_context(tc.tile_pool(name="wp", bufs=1))
        xp = ctx.enter_context(tc.tile_pool(name="xp", bufs=3))
        op = ctx.enter_context(tc.tile_pool(name="op", bufs=3))
        dp = ctx.enter_context(tc.tile_pool(name="dp", bufs=4))
        pp = ctx.enter_context(tc.tile_pool(name="pp", bufs=4, space="PSUM"))

        wmt = wp.tile([128, 256], BF16)
        nc.sync.dma_start(wmt[:], wm)
        wlt = wp.tile([128, 128], BF16)
        nc.sync.dma_start(wlt[:], wl)
        wrt = wp.tile([128, 128], BF16)
        nc.sync.dma_start(wrt[:], wr)

        for pr in range(8):          # sample-pair u = 2*pr, 2*pr+1 (same g)
            g = (2 * pr) // 4
            xts = []
            for h in range(2):
                u = 2 * pr + h
                xt = xp.tile([128, J], BF16, name=f"xt{h}")
                nc.sync.dma_start(xt[:], x[u * 128:(u + 1) * 128, :])
                xts.append(xt)
            ot = op.tile([128, M], BF16)

            for q in range(NQ):
                c0 = q * NSUB
                ptA = pp.tile([128, NSUB], F32, name="ptA")
                ptD = pp.tile([128, NSUB], F32, name="ptD")
                for h in range(2):
                    xt = xts[h]
                    # main taps
                    nc.tensor.matmul(
                        ptA[h * 64:(h + 1) * 64, :],
                        wmt[:, g * 64:(g + 1) * 64],
                        xt[:, 1 + c0:1 + c0 + NSUB],
                        start=True, stop=True, skip_group_check=True,
                    )
                    # left edge (x view shifted -1): nonzero outs r{0,1}
                    nc.tensor.matmul(
                        ptD[h * 64:h * 64 + 32, :],
                        wlt[:, g * 32:(g + 1) * 32],
                        xt[:, c0:c0 + NSUB],
                        start=True, stop=True, skip_group_check=True,
                        tile_position=(0, h * 64),
                    )
                    # right edge (+1): nonzero outs r{14,15}
                    nc.tensor.matmul(
                        ptD[h * 64 + 32:(h + 1) * 64, :],
                        wrt[:, g * 32:(g + 1) * 32],
                        xt[:, 2 + c0:2 + c0 + NSUB],
                        start=True, stop=True, skip_group_check=True,
                        tile_position=(0, h * 64 + 32),
                    )
                # TensorTensor may read at most one PSUM operand: stage D in
                # SBUF via the Act engine, then add A (PSUM) + D (SBUF).
                ds = dp.tile([128, NSUB], F32, name="ds")
                nc.scalar.copy(ds[:], ptD[:])
                nc.vector.tensor_tensor(ot[:, c0:c0 + NSUB], ptA[:], ds[:],
                                        mybir.AluOpType.add)

            nc.scalar.dma_start(y[pr * 128:(pr + 1) * 128, :], ot[:])
    nc.compile()
    return nc


def build_weights(kernels):
    """wm [128, 256], wl [128, 128], wr [128, 128] (bf16).

    wm[PERM[ph]*8+c, g*64 + r*4+o] = k[o,c,t], ph = r+t-2 in [0,16).
    wl: out col g*32 + r*4+o (r in {0,1}), taps from ph' = 14+e of col m-1:
        wl[PERM[14+e]*8+c, g*32 + r*4+o] = k[o,c,e-r] for 0 <= e-r < W.
    wr: out col g*32 + 24 + (r-14)*4+o (r in {14,15}), ph' = e of col m+1:
        wr[PERM[e]*8+c, ...] = k[o,c,18+e-r] for valid t.
    """
    wm = np.zeros((128, 256), np.float32)
    wl = np.zeros((128, 128), np.float32)
    wr = np.zeros((128, 128), np.float32)
    for g, ker in enumerate(kernels):       # ker [4, 8, 5]
        for o in range(4):
            for c in range(8):
                for r in range(16):
                    for t in range(W):
                        ph = r + t - 2
                        if 0 <= ph < 16:
                            wm[PERM[ph] * 8 + c, g * 64 + r * 4 + o] = ker[o, c, t]
                for e in range(2):
                    for r in (0, 1):
                        t = e - r
                        if 0 <= t < W:
                            wl[PERM[14 + e] * 8 + c, g * 32 + r * 4 + o] = ker[o, c, t]
                    for r in (14, 15):
                        t = 18 + e - r
                        if 0 <= t < W:
                            wr[PERM[e] * 8 + c, g * 32 + 24 + (r - 14) * 4 + o] = ker[o, c, t]
    return (wm.astype(NP_BF16), wl.astype(NP_BF16), wr.astype(NP_BF16))


def marshal_x(x4):
    """[4, 32, L] -> [16*128, J] bf16: u = g*4+s, row PERM[ph]*8+c, col j
    holds position 16*(j-1)+ph (zero-padded)."""
    xp = np.zeros((4, D, L + 32), np.float32)
    xp[:, :, 16:16 + L] = x4
    xph = xp.reshape(4, D, J, 16)[:, :, :, INVPERM]            # [s, ch, j, p]
    xr = xph.reshape(4, 4, 8, J, 16).transpose(1, 0, 4, 2, 3)  # [g, s, p, c, j]
    return np.ascontiguousarray(xr.reshape(16 * 128, J)).astype(NP_BF16)


def unmarshal_y(Y):
    """[16*64, M] bf16 -> [4, 16*L] fp32. Row = u*64 + r*4 + o, u = g*4+s."""
    t = np.asarray(Y, dtype=np.float32).reshape(16, 16, 4, M)   # [u, r, o, m]
    t = t.transpose(0, 2, 3, 1).reshape(16, 4, L)               # [u, o, n]
    t = t.reshape(4, 4, 4, L).transpose(1, 0, 2, 3)             # [s, g, o, n]
    return np.ascontiguousarray(t.reshape(4, 16 * L))


_program_cache = {}

# Set PROFILE=True (e.g. from a test harness) to capture an NTFF profile;
# the BassKernelResults lands in LAST_RESULT.
PROFILE = False
LAST_RESULT = None


def kernel(batch_x, kernels0, kernels1, kernels2, kernels3):
    global LAST_RESULT
    batch_x = np.asarray(batch_x)
    kernels = [np.asarray(k) for k in (kernels0, kernels1, kernels2, kernels3)]
    wm, wl, wr = build_weights(kernels)

    if "nc" not in _program_cache:
        _program_cache["nc"] = build_program()
    nc = _program_cache["nc"]

    x = batch_x.reshape(B, D, L)
    in_maps = []
    for k in range(N_CORES):
        x4 = x[S * k:S * (k + 1)]
        in_maps.append({"x": marshal_x(x4), "wm": wm, "wl": wl, "wr": wr})
    res = run_bass_kernel_spmd(nc, in_maps, list(range(N_CORES)), trace=PROFILE)
    LAST_RESULT = res
    ys = [unmarshal_y(res.results[k]["y"]) for k in range(N_CORES)]
    return np.concatenate(ys, axis=0)
